# revision 1
# baseline (speedup 1.0000x reference)
"""Trainium2 Bass kernel for nn_LNon_37460704756094 (embedding_lookup).

Math (reference):
    d   = (data - mean(data)) / std(data, ddof=1) * scalei
    s   = sigmoid(d); t = tanh(d)
    theta = interp(theta_lut, s * 119)   # theta_lut = linspace(-pi, pi, 120)
    velo  = interp(velo_lut, |t| * 119)  # velo_lut  = linspace(0, 3, 120)
    val = d * exp(velo * sin(theta)) + velo * cos(theta)
    out = (val - mean(val)) / std(val, ddof=1) * scaleo

Both LUTs are affine in the index, so linear interpolation collapses to an
affine map of the (continuous) index:
    theta = th0 + (th119 - th0) * s        (exact for an affine LUT)
    velo  = (v119 - v0) * |t|  (+ v0, asserted ~0)
cos(theta) = sin(theta + pi/2), so everything becomes Sigmoid/Tanh/Abs/Sin/
Exp/Square activations + a few vector ops. The affine coefficients are read
from the actual `params` input on the host at call time.

Distribution: batch-sharded over 8 cores (4 batches each = [128, 32768] f32
per core, SBUF-resident). Global mean/std for both normalizations via
per-partition accumulation -> partition_all_reduce -> 8-core AllReduce of a
[128, 2] stats buffer. HBM traffic is one 16 MiB read + one 16 MiB write
per core.
"""

import math

import numpy as np

import concourse.bacc as bacc
import concourse.bass as bass
import concourse.mybir as mybir
import concourse.tile as tile
from concourse.bass_utils import run_bass_kernel_spmd

N_CORES = 8
P = 128
B_FULL, C, H, W = 32, 64, 128, 128
PER_CORE = B_FULL // N_CORES * C * H * W          # 4,194,304
FREE = PER_CORE // P                              # 32,768
F = 1024                                          # tile free size
NT = FREE // F                                    # 32 tiles
N_TOTAL = B_FULL * C * H * W                      # 33,554,432

AF = mybir.ActivationFunctionType
ALU = mybir.AluOpType
AX = mybir.AxisListType
F32 = mybir.dt.float32

LAST_RESULT = None  # BassKernelResults of the most recent run (for test.py)

_KERNEL_CACHE = {}


def _build(consts, sim_mode=False):
    """Build the SPMD Bass program. `consts` = (th0, th_slope, v_slope)."""
    th0, th_slope, v_slope = consts
    halfpi = math.pi / 2.0

    nc = bacc.Bacc(None, num_devices=N_CORES)

    # Register the Sin biases as const APs (activation float biases are
    # looked up in nc.const_aps). Same pattern as Bass.__init__.
    for cv in (th0, th0 + halfpi):
        if (F32, cv) not in nc.const_aps.aps:
            t = nc.alloc_sbuf_tensor(f"const-f32-{cv}", [P, 1], F32)
            nc.gpsimd.memset(t.ap(), cv)
            nc.const_aps.aps[(F32, cv)] = t.ap()
    nc.all_engine_barrier()

    data_in = nc.dram_tensor("data", [P, FREE], F32, kind="ExternalInput")
    scal_in = nc.dram_tensor("scal", [P, 2], F32, kind="ExternalInput")
    out_dram = nc.dram_tensor("out", [P, FREE], F32, kind="ExternalOutput")

    groups = [list(range(N_CORES))]

    with tile.TileContext(nc) as tc:
        with (
            tc.tile_pool(name="big", bufs=1) as bigpool,
            tc.tile_pool(name="scr", bufs=3) as scr,
            tc.tile_pool(name="small", bufs=1) as smallpool,
            tc.tile_pool(name="psum", bufs=1, space="PSUM") as psumpool,
            tc.tile_pool(name="dram", bufs=1, space="DRAM") as dram,
        ):
            bigs = [bigpool.tile([P, F], F32, name=f"big{j}", tag=f"big{j}") for j in range(NT)]
            # per-tile partial stats: cols [0:NT) sum(x), [NT:2NT) sum(x^2),
            # [2NT:3NT) sum(val), [3NT:4NT) sum(val^2)
            statbuf = smallpool.tile([P, 4 * NT], F32, name="statbuf", tag="statbuf")
            # small scalars; phase A uses cols 0..15, phase B cols 16..31
            sm = smallpool.tile([P, 32], F32, name="sm", tag="sm")
            stA = smallpool.tile([P, 2], F32, name="stA", tag="stA")
            stB = smallpool.tile([P, 2], F32, name="stB", tag="stB")
            scal_all = smallpool.tile([P, 2], F32, name="scal_all", tag="scal_all")
            ones = smallpool.tile([P, P], F32, name="ones", tag="ones")
            psumA = psumpool.tile([P, 2], F32, name="psumA", tag="psumA")
            psumB = psumpool.tile([P, 2], F32, name="psumB", tag="psumB")

            cc_a_in = dram.tile([P, 2], F32, name="cc_a_in", tag="cc_a_in")
            cc_a_out = dram.tile([P, 2], F32, name="cc_a_out", tag="cc_a_out")
            cc_b_in = dram.tile([P, 2], F32, name="cc_b_in", tag="cc_b_in")
            cc_b_out = dram.tile([P, 2], F32, name="cc_b_out", tag="cc_b_out")

            # scalei / scaleo come pre-broadcast from the host as [128, 2]
            nc.gpsimd.dma_start(scal_all[:], scal_in[:])
            nc.vector.memset(ones[:], 1.0)

            # ---------------- Phase A: load + input stats ----------------
            for j in range(NT):
                sl = slice(j * F, (j + 1) * F)
                nc.sync.dma_start(bigs[j][:], data_in[:, sl])
                sq = scr.tile([P, F], F32, name="sq", tag="p")
                nc.scalar.activation(
                    sq[:], bigs[j][:], AF.Square,
                    accum_out=statbuf[:, NT + j : NT + j + 1],
                )
                nc.vector.reduce_sum(
                    statbuf[:, j : j + 1], bigs[j][:], axis=AX.X
                )

            nc.vector.reduce_sum(stA[:, 0:1], statbuf[:, 0:NT], axis=AX.X)
            nc.vector.reduce_sum(stA[:, 1:2], statbuf[:, NT : 2 * NT], axis=AX.X)

            # cross-core AllReduce of the [128, 2] per-partition partials
            nc.gpsimd.dma_start(cc_a_in[:], stA[:])
            if sim_mode:
                nc.gpsimd.dma_start(cc_a_out[:], cc_a_in[:])
            else:
                nc.gpsimd.collective_compute(
                    "AllReduce", ALU.add, replica_groups=groups,
                    ins=[cc_a_in.opt()], outs=[cc_a_out.opt()],
                )
            nc.gpsimd.dma_start(stA[:], cc_a_out[:])
            # ones.T @ stA: reduces across partitions AND broadcasts the
            # totals to every partition in one idle-PE matmul
            nc.tensor.matmul(psumA[:], ones[:], stA[:])
            nc.vector.tensor_copy(sm[:, 0:2], psumA[:])

            # a = scalei / std, b = -mean * a   (std unbiased, ddof=1)
            nc.vector.tensor_scalar_mul(sm[:, 2:3], sm[:, 0:1], 1.0 / N_TOTAL)   # mean
            nc.vector.tensor_mul(sm[:, 3:4], sm[:, 0:1], sm[:, 2:3])             # S1*mean
            nc.vector.tensor_sub(sm[:, 4:5], sm[:, 1:2], sm[:, 3:4])
            nc.vector.tensor_scalar_mul(sm[:, 5:6], sm[:, 4:5], 1.0 / (N_TOTAL - 1))
            nc.scalar.activation(sm[:, 6:7], sm[:, 5:6], AF.Sqrt)                # std
            nc.vector.reciprocal(sm[:, 7:8], sm[:, 6:7])                         # 1/std
            nc.vector.tensor_mul(sm[:, 8:9], sm[:, 7:8], scal_all[:, 0:1])      # a
            nc.vector.tensor_mul(sm[:, 9:10], sm[:, 2:3], sm[:, 8:9])
            nc.vector.tensor_scalar_mul(sm[:, 10:11], sm[:, 9:10], -1.0)         # b
            a_ap = sm[:, 8:9]
            b_ap = sm[:, 10:11]

            # ---------------- Phase B: elementwise chain + val stats -----
            for j in range(NT):
                d = bigs[j][:]
                s_ = scr.tile([P, F], F32, name="s", tag="s")
                t_ = scr.tile([P, F], F32, name="t", tag="t")
                u_ = scr.tile([P, F], F32, name="u", tag="u", bufs=2)
                T3 = scr.tile([P, F], F32, name="T3", tag="T3", bufs=2)
                sin_ = scr.tile([P, F], F32, name="sin", tag="sin")
                cos_ = scr.tile([P, F], F32, name="cos", tag="cos")
                p_ = scr.tile([P, F], F32, name="p", tag="p")

                nc.scalar.activation(s_[:], d, AF.Sigmoid, bias=b_ap, scale=a_ap)
                nc.scalar.activation(t_[:], d, AF.Tanh, bias=b_ap, scale=a_ap)
                nc.vector.tensor_scalar(
                    u_[:], d, a_ap, b_ap, op0=ALU.mult, op1=ALU.add
                )
                nc.scalar.activation(T3[:], t_[:], AF.Abs, scale=v_slope)
                nc.scalar.activation(sin_[:], s_[:], AF.Sin, bias=th0, scale=th_slope)
                nc.scalar.activation(
                    cos_[:], s_[:], AF.Sin, bias=th0 + halfpi, scale=th_slope
                )
                nc.vector.tensor_mul(p_[:], T3[:], sin_[:])
                nc.scalar.activation(sin_[:], p_[:], AF.Exp)                 # e
                nc.vector.tensor_mul(cos_[:], T3[:], cos_[:])                # q
                nc.vector.tensor_mul(u_[:], u_[:], sin_[:])                  # r = u*e
                nc.vector.tensor_add(d, u_[:], cos_[:])                      # val
                nc.scalar.activation(
                    t_[:], d, AF.Square,
                    accum_out=statbuf[:, 3 * NT + j : 3 * NT + j + 1],
                )
                nc.vector.reduce_sum(
                    statbuf[:, 2 * NT + j : 2 * NT + j + 1], d, axis=AX.X
                )

            nc.vector.reduce_sum(stB[:, 0:1], statbuf[:, 2 * NT : 3 * NT], axis=AX.X)
            nc.vector.reduce_sum(stB[:, 1:2], statbuf[:, 3 * NT : 4 * NT], axis=AX.X)

            nc.gpsimd.dma_start(cc_b_in[:], stB[:])
            if sim_mode:
                nc.gpsimd.dma_start(cc_b_out[:], cc_b_in[:])
            else:
                nc.gpsimd.collective_compute(
                    "AllReduce", ALU.add, replica_groups=groups,
                    ins=[cc_b_in.opt()], outs=[cc_b_out.opt()],
                )
            nc.gpsimd.dma_start(stB[:], cc_b_out[:])
            nc.tensor.matmul(psumB[:], ones[:], stB[:])
            nc.vector.tensor_copy(sm[:, 16:18], psumB[:])

            nc.vector.tensor_scalar_mul(sm[:, 18:19], sm[:, 16:17], 1.0 / N_TOTAL)
            nc.vector.tensor_mul(sm[:, 19:20], sm[:, 16:17], sm[:, 18:19])
            nc.vector.tensor_sub(sm[:, 20:21], sm[:, 17:18], sm[:, 19:20])
            nc.vector.tensor_scalar_mul(sm[:, 21:22], sm[:, 20:21], 1.0 / (N_TOTAL - 1))
            nc.scalar.activation(sm[:, 22:23], sm[:, 21:22], AF.Sqrt)
            nc.vector.reciprocal(sm[:, 23:24], sm[:, 22:23])
            nc.vector.tensor_mul(sm[:, 24:25], sm[:, 23:24], scal_all[:, 1:2])  # a2
            nc.vector.tensor_mul(sm[:, 25:26], sm[:, 18:19], sm[:, 24:25])
            nc.vector.tensor_scalar_mul(sm[:, 26:27], sm[:, 25:26], -1.0)        # b2
            a2_ap = sm[:, 24:25]
            b2_ap = sm[:, 26:27]

            # ---------------- Phase C: normalize + store -----------------
            for j in range(NT):
                sl = slice(j * F, (j + 1) * F)
                o_ = scr.tile([P, F], F32, name="o", tag="s")
                nc.vector.tensor_scalar(
                    o_[:], bigs[j][:], a2_ap, b2_ap, op0=ALU.mult, op1=ALU.add
                )
                nc.sync.dma_start(out_dram[:, sl], o_[:])

    nc.finalize()
    return nc


def kernel(data, params, scalei, scaleo):
    global LAST_RESULT
    data = np.ascontiguousarray(np.asarray(data, dtype=np.float32))
    params = np.asarray(params, dtype=np.float32)

    # Affine-LUT coefficients from the actual params input.
    th_lut = params[0, 0]
    v_lut = params[1, 0]
    npts = th_lut.shape[0]
    th0 = float(th_lut[0])
    th_slope = float(th_lut[npts - 1]) - th0
    v0 = float(v_lut[0])
    v_slope = float(v_lut[npts - 1]) - v0
    assert abs(v0) < 1e-6, f"velocity LUT must start at 0 (got {v0})"

    consts = (th0, th_slope, v_slope)
    nc = _KERNEL_CACHE.get(consts)
    if nc is None:
        nc = _build(consts)
        _KERNEL_CACHE[consts] = nc

    scal = np.tile(
        np.array(
            [[float(np.asarray(scalei).reshape(-1)[0]),
              float(np.asarray(scaleo).reshape(-1)[0])]],
            dtype=np.float32,
        ),
        (P, 1),
    )

    bpc = B_FULL // N_CORES
    in_maps = []
    for i in range(N_CORES):
        shard = np.ascontiguousarray(
            data[i * bpc : (i + 1) * bpc]
        ).reshape(P, FREE)
        in_maps.append({"data": shard, "scal": scal})

    res = run_bass_kernel_spmd(nc, in_maps, core_ids=list(range(N_CORES)))
    LAST_RESULT = res

    out = np.concatenate(
        [r["out"].reshape(bpc, C, H, W) for r in res.results], axis=0
    )
    return out



# revision 4
# speedup vs baseline: 1.1642x; 1.1642x over previous
"""Trainium2 Bass kernel for nn_LNon_37460704756094 (embedding_lookup).

Math (reference):
    d   = (data - mean(data)) / std(data, ddof=1) * scalei
    s   = sigmoid(d); t = tanh(d)
    theta = interp(theta_lut, s * 119)   # theta_lut = linspace(-pi, pi, 120)
    velo  = interp(velo_lut, |t| * 119)  # velo_lut  = linspace(0, 3, 120)
    val = d * exp(velo * sin(theta)) + velo * cos(theta)
    out = (val - mean(val)) / std(val, ddof=1) * scaleo

Both LUTs are affine in the index, so interpolation collapses to an affine
map.  theta = th0 + th_slope * sigmoid(d); with the half-angle identity
sigmoid(d) = (1 + tanh(d/2))/2 this becomes
    theta = (th0 + th_slope/2) + (th_slope/2) * tanh(d/2)
(for the reference LUT: theta = pi * tanh(d/2)).  This keeps every
activation needed per element inside just TWO hardware table sets:
  silu_and_others: tanh + sin        exp_and_others: exp (+ square/copy)
so the scalar engine does 5 passes (tanh, sin, sin, tanh, exp) with ~8
table loads total, instead of the 3-sets-per-tile thrash (99 loads) of the
sigmoid formulation.

Pipeline per core (data shard [128, 32768] f32):
  A: stream x in chunks; scalar Copy+accum converts to resident fp16 x16
     and yields sum(x); fused DVE tensor-scalar gives sum(x^2).
  AR1: 8-core AllReduce of [128,2] partial sums (a dummy AllReduce at
     kernel start absorbs the ring-setup latency).
  B (silu set): t=tanh((a x + b)/2) -> sin -> p,q slices (fp16);
     T=tanh(a x + b); T3=3|T| via abs_max; p*=T3, q*=T3 (fused TSP 4x ops).
  C (exp set):  e=exp(p); val = (a x16 + b)*e + q overwriting p, with
     sum(val)/sum(val^2) accumulated for free via STT accum_out.
     B and C are interleaved in quarters so C's vector work hides under
     the next quarter's scalar work (2 table loads per quarter).
  AR2, then D: out = a2*val + b2 streamed back to HBM in fp32.

fp16 intermediates keep the relative error ~1e-3 (gate is 2e-2).
"""

import math

import numpy as np

import concourse.bacc as bacc
import concourse.bass as bass
import concourse.mybir as mybir
import concourse.tile as tile
from concourse.bass_utils import run_bass_kernel_spmd

N_CORES = 8
P = 128
B_FULL, C, H, W = 32, 64, 128, 128
PER_CORE = B_FULL // N_CORES * C * H * W          # 4,194,304
FREE = PER_CORE // P                              # 32,768
N_TOTAL = B_FULL * C * H * W                      # 33,554,432

CA = 1024                                         # phase-A chunk
NA = FREE // CA                                   # 32
CB = 1024                                         # phase-B/C chunk
NB = FREE // CB                                   # 32
NQ = 4                                            # quarters for B/C interleave
CD = 1024                                         # phase-D chunk
ND = FREE // CD                                   # 32

AF = mybir.ActivationFunctionType
ALU = mybir.AluOpType
AX = mybir.AxisListType
F32 = mybir.dt.float32
F16 = mybir.dt.float16

LAST_RESULT = None  # BassKernelResults of the most recent run (for test.py)

_KERNEL_CACHE = {}


def _build(consts, sim_mode=False):
    """Build the SPMD Bass program.

    consts = (th_mid, th_half, v_slope):
      theta = th_mid + th_half * tanh(d/2);  velo = v_slope * |tanh(d)|
    """
    th_mid, th_half, v_slope = consts
    halfpi = math.pi / 2.0

    nc = bacc.Bacc(None, num_devices=N_CORES)

    # Activation float biases are looked up in nc.const_aps; register the
    # two Sin biases (same pattern as Bass.__init__).
    for cv in (th_mid, th_mid + halfpi):
        if (F32, cv) not in nc.const_aps.aps:
            t = nc.alloc_sbuf_tensor(f"const-f32-{cv}", [P, 1], F32)
            nc.gpsimd.memset(t.ap(), cv)
            nc.const_aps.aps[(F32, cv)] = t.ap()
    nc.all_engine_barrier()

    data_in = nc.dram_tensor("data", [P, FREE], F32, kind="ExternalInput")
    scal_in = nc.dram_tensor("scal", [P, 2], F32, kind="ExternalInput")
    out_dram = nc.dram_tensor("out", [P, FREE], F32, kind="ExternalOutput")

    groups = [list(range(N_CORES))]

    def all_reduce(cc_in, cc_out):
        if sim_mode:
            nc.gpsimd.dma_start(cc_out[:], cc_in[:])
        else:
            nc.gpsimd.collective_compute(
                "AllReduce", ALU.add, replica_groups=groups,
                ins=[cc_in.opt()], outs=[cc_out.opt()],
            )

    with tile.TileContext(nc) as tc:
        with (
            tc.tile_pool(name="keep", bufs=1) as keep,
            tc.tile_pool(name="psum", bufs=1, space="PSUM") as psumpool,
            tc.tile_pool(name="dram", bufs=1, space="DRAM") as dram,
        ):
            # ------- persistent SBUF (192 KiB/partition) -------
            x16 = keep.tile([P, FREE], F16, name="x16", tag="x16")
            pb = keep.tile([P, FREE], F16, name="pb", tag="pb")
            qb = keep.tile([P, FREE], F16, name="qb", tag="qb")
            # ------- small persistent -------
            statA = keep.tile([P, 2 * NA], F32, name="statA", tag="statA")
            statC = keep.tile([P, 2 * NB], F32, name="statC", tag="statC")
            sm = keep.tile([P, 32], F32, name="sm", tag="sm")
            stA = keep.tile([P, 2], F32, name="stA", tag="stA")
            stB = keep.tile([P, 2], F32, name="stB", tag="stB")
            scal_all = keep.tile([P, 2], F32, name="scal_all", tag="scal_all")
            ones = keep.tile([P, P], F32, name="ones", tag="ones")
            psumA = psumpool.tile([P, 2], F32, name="psumA", tag="psumA")
            psumB = psumpool.tile([P, 2], F32, name="psumB", tag="psumB")

            cc_w_in = dram.tile([P, 2], F32, name="cc_w_in", tag="cc_w_in")
            cc_w_out = dram.tile([P, 2], F32, name="cc_w_out", tag="cc_w_out")
            cc_a_in = dram.tile([P, 2], F32, name="cc_a_in", tag="cc_a_in")
            cc_a_out = dram.tile([P, 2], F32, name="cc_a_out", tag="cc_a_out")
            cc_b_in = dram.tile([P, 2], F32, name="cc_b_in", tag="cc_b_in")
            cc_b_out = dram.tile([P, 2], F32, name="cc_b_out", tag="cc_b_out")

            # Dummy AllReduce issued first: absorbs the ring-setup latency
            # (~25us) so AR1/AR2 run at steady-state (~10us).
            all_reduce(cc_w_in, cc_w_out)

            nc.gpsimd.dma_start(scal_all[:], scal_in[:])
            nc.vector.memset(ones[:], 1.0)

            # ---------------- Phase A: load + convert + input stats ------
            with tc.tile_pool(name="pa", bufs=1) as pa:
                xin = [
                    pa.tile([P, CA], F32, name=f"xin{i}", tag=f"xin{i}")
                    for i in range(2)
                ]
                sqa = pa.tile([P, CA], F16, name="sqa", tag="sqa")
                for j in range(NA):
                    sl = slice(j * CA, (j + 1) * CA)
                    xb = xin[j % 2]
                    nc.sync.dma_start(xb[:], data_in[:, sl])
                    # fp32 -> fp16 convert + per-partition sum(x), one pass
                    nc.scalar.activation(
                        x16[:, sl], xb[:], AF.Copy,
                        accum_out=statA[:, j : j + 1],
                    )
                    # sum(x^2): out = (x * 1) * x (discarded), accum = sum
                    nc.vector.scalar_tensor_tensor(
                        sqa[:], xb[:], 1.0, xb[:],
                        op0=ALU.mult, op1=ALU.mult,
                        accum_out=statA[:, NA + j : NA + j + 1],
                    )

                nc.vector.reduce_sum(stA[:, 0:1], statA[:, 0:NA], axis=AX.X)
                nc.vector.reduce_sum(stA[:, 1:2], statA[:, NA : 2 * NA], axis=AX.X)

            # cross-core AllReduce of the [128, 2] per-partition partials
            nc.gpsimd.dma_start(cc_a_in[:], stA[:])
            all_reduce(cc_a_in, cc_a_out)
            nc.gpsimd.dma_start(stA[:], cc_a_out[:])
            # ones.T @ stA reduces across partitions AND broadcasts totals
            nc.tensor.matmul(psumA[:], ones[:], stA[:])
            nc.vector.tensor_copy(sm[:, 0:2], psumA[:])

            # a = scalei / std, b = -mean * a   (std unbiased, ddof=1)
            nc.vector.tensor_scalar_mul(sm[:, 2:3], sm[:, 0:1], 1.0 / N_TOTAL)  # mean
            nc.vector.tensor_mul(sm[:, 3:4], sm[:, 0:1], sm[:, 2:3])            # S1*mean
            nc.vector.tensor_sub(sm[:, 4:5], sm[:, 1:2], sm[:, 3:4])
            nc.vector.tensor_scalar_mul(sm[:, 5:6], sm[:, 4:5], 1.0 / (N_TOTAL - 1))
            nc.scalar.activation(sm[:, 6:7], sm[:, 5:6], AF.Sqrt)               # std
            nc.vector.reciprocal(sm[:, 7:8], sm[:, 6:7])                        # 1/std
            nc.vector.tensor_mul(sm[:, 8:9], sm[:, 7:8], scal_all[:, 0:1])      # a
            nc.vector.tensor_mul(sm[:, 9:10], sm[:, 2:3], sm[:, 8:9])
            nc.vector.tensor_scalar_mul(sm[:, 10:11], sm[:, 9:10], -1.0)        # b
            nc.vector.tensor_scalar_mul(sm[:, 11:12], sm[:, 8:9], 0.5)          # a/2
            nc.vector.tensor_scalar_mul(sm[:, 12:13], sm[:, 10:11], 0.5)        # b/2
            a_ap = sm[:, 8:9]
            b_ap = sm[:, 10:11]
            ah_ap = sm[:, 11:12]
            bh_ap = sm[:, 12:13]

            # ---------- Phases B (silu set) + C (exp set), interleaved ----
            with tc.tile_pool(name="pb_scr", bufs=1) as pbs:
                t_ = pbs.tile([P, CB], F16, name="t_", tag="t_")
                T_ = pbs.tile([P, CB], F16, name="T_", tag="T_")
                T3 = pbs.tile([P, CB], F16, name="T3", tag="T3")
                ee = [
                    pbs.tile([P, CB], F16, name=f"ee{i}", tag=f"ee{i}")
                    for i in range(2)
                ]
                uu = pbs.tile([P, CB], F16, name="uu", tag="uu")

                per_q = NB // NQ
                for qq in range(NQ):
                    chunks = range(qq * per_q, (qq + 1) * per_q)
                    # --- B: silu-set activations + fp16 vector fusions ---
                    for j in chunks:
                        sl = slice(j * CB, (j + 1) * CB)
                        xs = x16[:, sl]
                        # t = tanh(d/2); theta = th_mid + th_half * t
                        nc.scalar.activation(t_[:], xs, AF.Tanh, bias=bh_ap, scale=ah_ap)
                        # sin(theta), cos(theta) written straight into p, q
                        nc.scalar.activation(pb[:, sl], t_[:], AF.Sin, bias=th_mid, scale=th_half)
                        nc.scalar.activation(qb[:, sl], t_[:], AF.Sin, bias=th_mid + halfpi, scale=th_half)
                        # T = tanh(d); T3 = |T| = max(-T, T)
                        nc.scalar.activation(T_[:], xs, AF.Tanh, bias=b_ap, scale=a_ap)
                        nc.vector.scalar_tensor_tensor(
                            T3[:], T_[:], -1.0, T_[:], op0=ALU.mult, op1=ALU.max
                        )
                        # p = v_slope*|T|*sin(theta), q = v_slope*|T|*cos(theta)
                        nc.vector.scalar_tensor_tensor(
                            pb[:, sl], pb[:, sl], v_slope, T3[:], op0=ALU.mult, op1=ALU.mult
                        )
                        nc.vector.scalar_tensor_tensor(
                            qb[:, sl], qb[:, sl], v_slope, T3[:], op0=ALU.mult, op1=ALU.mult
                        )
                    # --- C: exp-set + val assembly + val stats ---
                    for j in chunks:
                        sl = slice(j * CB, (j + 1) * CB)
                        e_ = ee[j % 2]
                        nc.scalar.activation(e_[:], pb[:, sl], AF.Exp)
                        # u = a*x + b
                        nc.vector.tensor_scalar(
                            uu[:], x16[:, sl], a_ap, b_ap, op0=ALU.mult, op1=ALU.add
                        )
                        # r = u*e  (overwrites p slice; val lives there)
                        nc.vector.scalar_tensor_tensor(
                            pb[:, sl], uu[:], 1.0, e_[:], op0=ALU.mult, op1=ALU.mult
                        )
                        # val = r + q, accum -> sum(val)
                        nc.vector.scalar_tensor_tensor(
                            pb[:, sl], pb[:, sl], 1.0, qb[:, sl],
                            op0=ALU.mult, op1=ALU.add,
                            accum_out=statC[:, j : j + 1],
                        )
                        # val^2 (discarded into uu), accum -> sum(val^2)
                        nc.vector.scalar_tensor_tensor(
                            uu[:], pb[:, sl], 1.0, pb[:, sl],
                            op0=ALU.mult, op1=ALU.mult,
                            accum_out=statC[:, NB + j : NB + j + 1],
                        )

                nc.vector.reduce_sum(stB[:, 0:1], statC[:, 0:NB], axis=AX.X)
                nc.vector.reduce_sum(stB[:, 1:2], statC[:, NB : 2 * NB], axis=AX.X)

            nc.gpsimd.dma_start(cc_b_in[:], stB[:])
            all_reduce(cc_b_in, cc_b_out)
            nc.gpsimd.dma_start(stB[:], cc_b_out[:])
            nc.tensor.matmul(psumB[:], ones[:], stB[:])
            nc.vector.tensor_copy(sm[:, 16:18], psumB[:])

            nc.vector.tensor_scalar_mul(sm[:, 18:19], sm[:, 16:17], 1.0 / N_TOTAL)
            nc.vector.tensor_mul(sm[:, 19:20], sm[:, 16:17], sm[:, 18:19])
            nc.vector.tensor_sub(sm[:, 20:21], sm[:, 17:18], sm[:, 19:20])
            nc.vector.tensor_scalar_mul(sm[:, 21:22], sm[:, 20:21], 1.0 / (N_TOTAL - 1))
            nc.scalar.activation(sm[:, 22:23], sm[:, 21:22], AF.Sqrt)
            nc.vector.reciprocal(sm[:, 23:24], sm[:, 22:23])
            nc.vector.tensor_mul(sm[:, 24:25], sm[:, 23:24], scal_all[:, 1:2])  # a2
            nc.vector.tensor_mul(sm[:, 25:26], sm[:, 18:19], sm[:, 24:25])
            nc.vector.tensor_scalar_mul(sm[:, 26:27], sm[:, 25:26], -1.0)       # b2
            a2_ap = sm[:, 24:25]
            b2_ap = sm[:, 26:27]

            # ---------------- Phase D: normalize + store -----------------
            with tc.tile_pool(name="pd", bufs=1) as pd:
                outs = [
                    pd.tile([P, CD], F32, name=f"o{i}", tag=f"o{i}")
                    for i in range(2)
                ]
                for j in range(ND):
                    sl = slice(j * CD, (j + 1) * CD)
                    o_ = outs[j % 2]
                    nc.vector.tensor_scalar(
                        o_[:], pb[:, sl], a2_ap, b2_ap, op0=ALU.mult, op1=ALU.add
                    )
                    nc.sync.dma_start(out_dram[:, sl], o_[:])

    nc.finalize()
    return nc


def kernel(data, params, scalei, scaleo):
    global LAST_RESULT
    data = np.ascontiguousarray(np.asarray(data, dtype=np.float32))
    params = np.asarray(params, dtype=np.float32)

    # Affine-LUT coefficients from the actual params input.
    th_lut = params[0, 0]
    v_lut = params[1, 0]
    npts = th_lut.shape[0]
    th0 = float(th_lut[0])
    th_slope = float(th_lut[npts - 1]) - th0
    v0 = float(v_lut[0])
    v_slope = float(v_lut[npts - 1]) - v0
    assert abs(v0) < 1e-6, f"velocity LUT must start at 0 (got {v0})"

    # theta = th0 + th_slope*sigmoid(d) = th_mid + th_half*tanh(d/2)
    th_mid = th0 + 0.5 * th_slope
    th_half = 0.5 * th_slope

    consts = (th_mid, th_half, v_slope)
    nc = _KERNEL_CACHE.get(consts)
    if nc is None:
        nc = _build(consts)
        _KERNEL_CACHE[consts] = nc

    scal = np.tile(
        np.array(
            [[float(np.asarray(scalei).reshape(-1)[0]),
              float(np.asarray(scaleo).reshape(-1)[0])]],
            dtype=np.float32,
        ),
        (P, 1),
    )

    bpc = B_FULL // N_CORES
    in_maps = []
    for i in range(N_CORES):
        shard = np.ascontiguousarray(
            data[i * bpc : (i + 1) * bpc]
        ).reshape(P, FREE)
        in_maps.append({"data": shard, "scal": scal})

    res = run_bass_kernel_spmd(nc, in_maps, core_ids=list(range(N_CORES)))
    LAST_RESULT = res

    out = np.concatenate(
        [r["out"].reshape(bpc, C, H, W) for r in res.results], axis=0
    )
    return out


# revision 7
# speedup vs baseline: 1.4228x; 1.2221x over previous
"""Trainium2 Bass kernel for nn_LNon_37460704756094 (embedding_lookup).

Math (reference):
    d   = (data - mean(data)) / std(data, ddof=1) * scalei
    s   = sigmoid(d); t = tanh(d)
    theta = interp(theta_lut, s * 119)   # theta_lut = linspace(-pi, pi, 120)
    velo  = interp(velo_lut, |t| * 119)  # velo_lut  = linspace(0, 3, 120)
    val = d * exp(velo * sin(theta)) + velo * cos(theta)
    out = (val - mean(val)) / std(val, ddof=1) * scaleo

Both LUTs are affine in the index, so interpolation collapses to an affine
map.  With sigmoid(d) = (1 + tanh(d/2))/2:
    theta = th_mid + th_half * tanh(d/2)        (= pi*tanh(d/2) here)
    velo  = v_slope * |tanh(d)|
Every scalar-engine activation then lives in just two table sets
(tanh/exp/square in one, sin in another), and sins are batched per quarter
so the set switch costs 2 loads/quarter instead of 2/chunk.

Pipeline per core (shard [128, 32768] f32):
  A: stream x (32 x 1024 chunks, 3 landing buffers); scalar Copy+accum
     converts to resident fp16 x16 and yields sum(x); vector fused
     tensor-scalar yields sum(x^2).  All hidden under the HBM read.
  AR1 (warmed by a dummy AllReduce at kernel start).
  Mid, in 4 quarters over 8 x 4096 chunks:
    [tanh set] t=tanh((a x+b)/2) -> pb
    [sin set]  qb=sin(th_half*t+th_mid+pi/2); pb=sin(th_half*t+th_mid) inplace
    [tanh set] T_=tanh(a x+b); T3=|T| (fused mult/max); pb*=T3; qb*=T3;
               x16 = a*x16+b in place (u)
    [exp set, 1024 sub-chunks] e=exp(v_slope*pb); e*=u (r, in place);
               val = v_slope*qb + r -> pb with accum sum(val);
               val^2 -> dead qb with accum sum(val^2)
  AR2, then D: out = a2*val + b2 streamed to HBM (3 staging buffers).

fp16 intermediates keep rel err ~1e-3 (gate 2e-2).
"""

import math

import numpy as np

import concourse.bacc as bacc
import concourse.bass as bass
import concourse.mybir as mybir
import concourse.tile as tile
from concourse.bass_utils import run_bass_kernel_spmd

N_CORES = 8
P = 128
B_FULL, C, H, W = 32, 64, 128, 128
PER_CORE = B_FULL // N_CORES * C * H * W          # 4,194,304
FREE = PER_CORE // P                              # 32,768
N_TOTAL = B_FULL * C * H * W                      # 33,554,432

CA = 1024                                         # phase-A chunk
NA = FREE // CA                                   # 32
CB = 4096                                         # tanh/sin chunk
NB = FREE // CB                                   # 8
NQ = 4                                            # quarters (2 chunks each)
CC = 1024                                         # exp/val chunk
NC = FREE // CC                                   # 32
CD = 1024                                         # store chunk
ND = FREE // CD                                   # 32

AF = mybir.ActivationFunctionType
ALU = mybir.AluOpType
AX = mybir.AxisListType
F32 = mybir.dt.float32
F16 = mybir.dt.float16

LAST_RESULT = None  # BassKernelResults of the most recent run (for test.py)

_KERNEL_CACHE = {}


def _build(consts, sim_mode=False):
    """consts = (th_mid, th_half, v_slope)."""
    th_mid, th_half, v_slope = consts
    halfpi = math.pi / 2.0

    nc = bacc.Bacc(None, num_devices=N_CORES)

    for cv in (th_mid, th_mid + halfpi):
        if (F32, cv) not in nc.const_aps.aps:
            t = nc.alloc_sbuf_tensor(f"const-f32-{cv}", [P, 1], F32)
            nc.gpsimd.memset(t.ap(), cv)
            nc.const_aps.aps[(F32, cv)] = t.ap()
    nc.all_engine_barrier()

    data_in = nc.dram_tensor("data", [P, FREE], F32, kind="ExternalInput")
    scal_in = nc.dram_tensor("scal", [P, 2], F32, kind="ExternalInput")
    out_dram = nc.dram_tensor("out", [P, FREE], F32, kind="ExternalOutput")

    groups = [list(range(N_CORES))]

    def all_reduce(cc_in, cc_out):
        if sim_mode:
            nc.gpsimd.dma_start(cc_out[:], cc_in[:])
        else:
            nc.gpsimd.collective_compute(
                "AllReduce", ALU.add, replica_groups=groups,
                ins=[cc_in.opt()], outs=[cc_out.opt()],
            )

    with tile.TileContext(nc) as tc:
        with (
            tc.tile_pool(name="keep", bufs=1) as keep,
            tc.tile_pool(name="psum", bufs=1, space="PSUM") as psumpool,
            tc.tile_pool(name="dram", bufs=1, space="DRAM") as dram,
        ):
            # ------- persistent SBUF (192 KiB/partition) -------
            x16 = keep.tile([P, FREE], F16, name="x16", tag="x16")
            pb = keep.tile([P, FREE], F16, name="pb", tag="pb")
            qb = keep.tile([P, FREE], F16, name="qb", tag="qb")
            # ------- small persistent -------
            statA = keep.tile([P, 2 * NA], F32, name="statA", tag="statA")
            statC = keep.tile([P, 2 * NC], F32, name="statC", tag="statC")
            sm = keep.tile([P, 32], F32, name="sm", tag="sm")
            stA = keep.tile([P, 2], F32, name="stA", tag="stA")
            stB = keep.tile([P, 2], F32, name="stB", tag="stB")
            scal_all = keep.tile([P, 2], F32, name="scal_all", tag="scal_all")
            ones = keep.tile([P, P], F32, name="ones", tag="ones")
            psumA = psumpool.tile([P, 2], F32, name="psumA", tag="psumA")
            psumB = psumpool.tile([P, 2], F32, name="psumB", tag="psumB")

            cc_w_in = dram.tile([P, 2], F32, name="cc_w_in", tag="cc_w_in")
            cc_w_out = dram.tile([P, 2], F32, name="cc_w_out", tag="cc_w_out")
            cc_a_in = dram.tile([P, 2], F32, name="cc_a_in", tag="cc_a_in")
            cc_a_out = dram.tile([P, 2], F32, name="cc_a_out", tag="cc_a_out")
            cc_b_in = dram.tile([P, 2], F32, name="cc_b_in", tag="cc_b_in")
            cc_b_out = dram.tile([P, 2], F32, name="cc_b_out", tag="cc_b_out")

            # Dummy AllReduce: absorbs collective ring-setup latency.
            all_reduce(cc_w_in, cc_w_out)

            nc.gpsimd.dma_start(scal_all[:], scal_in[:])
            nc.vector.memset(ones[:], 1.0)

            # ---------------- Phase A: load + convert + input stats ------
            with tc.tile_pool(name="pa", bufs=1) as pa:
                xin = [
                    pa.tile([P, CA], F32, name=f"xin{i}", tag=f"xin{i}")
                    for i in range(3)
                ]
                sqa = pa.tile([P, CA], F16, name="sqa", tag="sqa")
                for j in range(NA):
                    sl = slice(j * CA, (j + 1) * CA)
                    xb = xin[j % 3]
                    nc.sync.dma_start(xb[:], data_in[:, sl])
                    # fp32 -> fp16 convert + per-partition sum(x), one pass
                    nc.scalar.activation(
                        x16[:, sl], xb[:], AF.Copy,
                        accum_out=statA[:, j : j + 1],
                    )
                    # sum(x^2): out discarded, accum = sum
                    nc.vector.scalar_tensor_tensor(
                        sqa[:], xb[:], 1.0, xb[:],
                        op0=ALU.mult, op1=ALU.mult,
                        accum_out=statA[:, NA + j : NA + j + 1],
                    )

                nc.vector.reduce_sum(stA[:, 0:1], statA[:, 0:NA], axis=AX.X)
                nc.vector.reduce_sum(stA[:, 1:2], statA[:, NA : 2 * NA], axis=AX.X)

            nc.gpsimd.dma_start(cc_a_in[:], stA[:])
            all_reduce(cc_a_in, cc_a_out)
            nc.gpsimd.dma_start(stA[:], cc_a_out[:])
            nc.tensor.matmul(psumA[:], ones[:], stA[:])
            nc.vector.tensor_copy(sm[:, 0:2], psumA[:])

            # a = scalei / std, b = -mean * a   (std unbiased, ddof=1)
            nc.vector.tensor_scalar_mul(sm[:, 2:3], sm[:, 0:1], 1.0 / N_TOTAL)  # mean
            nc.vector.tensor_mul(sm[:, 3:4], sm[:, 0:1], sm[:, 2:3])            # S1*mean
            nc.vector.tensor_sub(sm[:, 4:5], sm[:, 1:2], sm[:, 3:4])
            nc.vector.tensor_scalar_mul(sm[:, 5:6], sm[:, 4:5], 1.0 / (N_TOTAL - 1))
            nc.scalar.activation(sm[:, 6:7], sm[:, 5:6], AF.Sqrt)               # std
            nc.vector.reciprocal(sm[:, 7:8], sm[:, 6:7])                        # 1/std
            nc.vector.tensor_mul(sm[:, 8:9], sm[:, 7:8], scal_all[:, 0:1])      # a
            nc.vector.tensor_mul(sm[:, 9:10], sm[:, 2:3], sm[:, 8:9])
            nc.vector.tensor_scalar_mul(sm[:, 10:11], sm[:, 9:10], -1.0)        # b
            nc.vector.tensor_scalar_mul(sm[:, 11:12], sm[:, 8:9], 0.5)          # a/2
            nc.vector.tensor_scalar_mul(sm[:, 12:13], sm[:, 10:11], 0.5)        # b/2
            a_ap = sm[:, 8:9]
            b_ap = sm[:, 10:11]
            ah_ap = sm[:, 11:12]
            bh_ap = sm[:, 12:13]

            # ---------- Mid: quarters of [tanh | sin | tanh+vec | exp+val]
            with tc.tile_pool(name="pm", bufs=1) as pm:
                T_ = pm.tile([P, CB], F16, name="T_", tag="T_")
                ee = [
                    pm.tile([P, CC], F16, name=f"ee{i}", tag=f"ee{i}")
                    for i in range(2)
                ]

                nbq = NB // NQ                      # chunks per quarter
                ncq = NC // NQ                      # exp sub-chunks per quarter
                for qq in range(NQ):
                    # --- t = tanh(d/2) -> pb  (tanh set) ---
                    for j in range(qq * nbq, (qq + 1) * nbq):
                        sl = slice(j * CB, (j + 1) * CB)
                        nc.scalar.activation(
                            pb[:, sl], x16[:, sl], AF.Tanh, bias=bh_ap, scale=ah_ap
                        )
                    # --- sin set: cos first (reads t), then sin in place ---
                    for j in range(qq * nbq, (qq + 1) * nbq):
                        sl = slice(j * CB, (j + 1) * CB)
                        nc.scalar.activation(
                            qb[:, sl], pb[:, sl], AF.Sin,
                            bias=th_mid + halfpi, scale=th_half,
                        )
                        nc.scalar.activation(
                            pb[:, sl], pb[:, sl], AF.Sin,
                            bias=th_mid, scale=th_half,
                        )
                    # --- tanh set: T, |T|, p, q, u ---
                    for j in range(qq * nbq, (qq + 1) * nbq):
                        sl = slice(j * CB, (j + 1) * CB)
                        nc.scalar.activation(
                            T_[:], x16[:, sl], AF.Tanh, bias=b_ap, scale=a_ap
                        )
                        nc.vector.scalar_tensor_tensor(
                            T_[:], T_[:], -1.0, T_[:], op0=ALU.mult, op1=ALU.max
                        )
                        nc.vector.tensor_mul(pb[:, sl], pb[:, sl], T_[:])
                        nc.vector.tensor_mul(qb[:, sl], qb[:, sl], T_[:])
                        # u = a*x + b, in place over x16
                        nc.vector.tensor_scalar(
                            x16[:, sl], x16[:, sl], a_ap, b_ap,
                            op0=ALU.mult, op1=ALU.add,
                        )
                    # --- exp set: e, r, val (+stats) ---
                    for j in range(qq * ncq, (qq + 1) * ncq):
                        sl = slice(j * CC, (j + 1) * CC)
                        e_ = ee[j % 2]
                        nc.scalar.activation(e_[:], pb[:, sl], AF.Exp, scale=v_slope)
                        # r = u * e  (in place over e)
                        nc.vector.tensor_mul(e_[:], x16[:, sl], e_[:])
                        # val = v_slope*q + r -> pb, accum sum(val)
                        nc.vector.scalar_tensor_tensor(
                            pb[:, sl], qb[:, sl], v_slope, e_[:],
                            op0=ALU.mult, op1=ALU.add,
                            accum_out=statC[:, j : j + 1],
                        )
                        # val^2 -> dead qb, accum sum(val^2)
                        nc.vector.scalar_tensor_tensor(
                            qb[:, sl], pb[:, sl], 1.0, pb[:, sl],
                            op0=ALU.mult, op1=ALU.mult,
                            accum_out=statC[:, NC + j : NC + j + 1],
                        )

                nc.vector.reduce_sum(stB[:, 0:1], statC[:, 0:NC], axis=AX.X)
                nc.vector.reduce_sum(stB[:, 1:2], statC[:, NC : 2 * NC], axis=AX.X)

            nc.gpsimd.dma_start(cc_b_in[:], stB[:])
            all_reduce(cc_b_in, cc_b_out)
            nc.gpsimd.dma_start(stB[:], cc_b_out[:])
            nc.tensor.matmul(psumB[:], ones[:], stB[:])
            nc.vector.tensor_copy(sm[:, 16:18], psumB[:])

            nc.vector.tensor_scalar_mul(sm[:, 18:19], sm[:, 16:17], 1.0 / N_TOTAL)
            nc.vector.tensor_mul(sm[:, 19:20], sm[:, 16:17], sm[:, 18:19])
            nc.vector.tensor_sub(sm[:, 20:21], sm[:, 17:18], sm[:, 19:20])
            nc.vector.tensor_scalar_mul(sm[:, 21:22], sm[:, 20:21], 1.0 / (N_TOTAL - 1))
            nc.scalar.activation(sm[:, 22:23], sm[:, 21:22], AF.Sqrt)
            nc.vector.reciprocal(sm[:, 23:24], sm[:, 22:23])
            nc.vector.tensor_mul(sm[:, 24:25], sm[:, 23:24], scal_all[:, 1:2])  # a2
            nc.vector.tensor_mul(sm[:, 25:26], sm[:, 18:19], sm[:, 24:25])
            nc.vector.tensor_scalar_mul(sm[:, 26:27], sm[:, 25:26], -1.0)       # b2
            a2_ap = sm[:, 24:25]
            b2_ap = sm[:, 26:27]

            # ---------------- Phase D: normalize + store -----------------
            with tc.tile_pool(name="pd", bufs=1) as pd:
                outs = [
                    pd.tile([P, CD], F32, name=f"o{i}", tag=f"o{i}")
                    for i in range(3)
                ]
                for j in range(ND):
                    sl = slice(j * CD, (j + 1) * CD)
                    o_ = outs[j % 3]
                    nc.vector.tensor_scalar(
                        o_[:], pb[:, sl], a2_ap, b2_ap, op0=ALU.mult, op1=ALU.add
                    )
                    nc.sync.dma_start(out_dram[:, sl], o_[:])

    nc.finalize()
    return nc


def kernel(data, params, scalei, scaleo):
    global LAST_RESULT
    data = np.ascontiguousarray(np.asarray(data, dtype=np.float32))
    params = np.asarray(params, dtype=np.float32)

    # Affine-LUT coefficients from the actual params input.
    th_lut = params[0, 0]
    v_lut = params[1, 0]
    npts = th_lut.shape[0]
    th0 = float(th_lut[0])
    th_slope = float(th_lut[npts - 1]) - th0
    v0 = float(v_lut[0])
    v_slope = float(v_lut[npts - 1]) - v0
    assert abs(v0) < 1e-6, f"velocity LUT must start at 0 (got {v0})"

    # theta = th0 + th_slope*sigmoid(d) = th_mid + th_half*tanh(d/2)
    th_mid = th0 + 0.5 * th_slope
    th_half = 0.5 * th_slope

    consts = (th_mid, th_half, v_slope)
    nc = _KERNEL_CACHE.get(consts)
    if nc is None:
        nc = _build(consts)
        _KERNEL_CACHE[consts] = nc

    scal = np.tile(
        np.array(
            [[float(np.asarray(scalei).reshape(-1)[0]),
              float(np.asarray(scaleo).reshape(-1)[0])]],
            dtype=np.float32,
        ),
        (P, 1),
    )

    bpc = B_FULL // N_CORES
    in_maps = []
    for i in range(N_CORES):
        shard = np.ascontiguousarray(
            data[i * bpc : (i + 1) * bpc]
        ).reshape(P, FREE)
        in_maps.append({"data": shard, "scal": scal})

    res = run_bass_kernel_spmd(nc, in_maps, core_ids=list(range(N_CORES)))
    LAST_RESULT = res

    out = np.concatenate(
        [r["out"].reshape(bpc, C, H, W) for r in res.results], axis=0
    )
    return out
